# revision 24
# baseline (speedup 1.0000x reference)
"""Bidirectional LSTM encoder (nn_BiEncode) as a Bass/Tile kernel on 8 trn2 cores.

Sharding: direction-split x batch-split. Cores 0-3 run the LEFT (forward-time)
direction on batch shards 0-3 (512 rows each); cores 4-7 run the RIGHT
direction (time-reversed input, handled host-side) on the same batch shards.
Every core runs the identical SPMD program; direction differences live
entirely in the data it is fed (weights + time-reversed x).

Device layout: everything is kept "transposed" (feature dim on partitions,
batch on the free dim) so the scan needs no on-chip transposes:
  x fed as xP[t, p, k, b], weights as W^T (j-major, partition-contiguous),
  h/c as [H, B] tiles, output written as yT[t, h, b] and un-transposed on
  the host.

Per timestep the full gate pre-activation g^T[4H, B] is computed as 12
PSUM-accumulated matmuls per 128-row gate tile (8 k-tiles of x-projection +
4 k-tiles of the recurrent term) -- the input projection is fused into the
scan, so no pre-activation tensor is ever materialized. ACT applies
sigmoid/tanh straight out of PSUM; DVE does the cell update.

The matmul stream runs at the bf16 PE floor (N/2.4GHz + 2.5ns = 215.8ns per
128x128x512 MM; 4928 MMs = 1063.6us), so the only tunable time is the
startup ramp (DMA of the first 2MB of x0/w_ih[j0]) and the terminal
drain/teardown:
  - All host-side tensors are laid out so each SBUF tile's per-partition
    bytes are CONTIGUOUS in DRAM (8KB rows for full k-tiles, 2KB rows for
    the t=0 pair slices) -- bigger DMA packets, ~1.4x ring bandwidth vs the
    1KB-row patterns a device-side rearrange produces.
  - t=0's x and w_ih[j0] load as 2-k-slice "pair" tiles interleaved across
    the sync and scalar queues so both streams land k-ordered; w_ih[j1..j3]
    follow, each split in half across the queues (j1 is the binding
    constraint for a stall-free stream); w_hh and x1 ride behind on scalar.
  - The PE_HAM warmup (see below) is trimmed to end right when the DMA
    pacing allows the real stream to run stall-free.
  - fp8 (e4m3/e5m2 + DoubleRow, 1.44x) was evaluated and is numerically
    DEAD for this problem: simulated end-to-end rel-err 0.08-0.16 vs the
    2e-2 budget (bf16 sits at 9.6e-3). Don't go back down that road.
"""

import os

import numpy as np

FRAME_LENGTH = 26
HIDDEN = 512
INPUT = 1024
BATCH = 2048

NCORES = 8
NSHARD = 4                 # batch shards per direction group
BC = BATCH // NSHARD       # 512 batch rows per core

P = 128
KI = INPUT // P            # 8  k-tiles for the input projection
KH = HIDDEN // P           # 4  k-tiles for the recurrent matmul
NJ = HIDDEN // P           # 4  hidden chunks
NM = 4 * HIDDEN // P       # 16 gate m-tiles

# "f32r": fp32 storage, PE in float32r (full-rate at N>=256, ~tf32
#          precision).  227 ns/MM sustained, rel err ~6e-4.
# "bf16": bf16 storage+PE (half DMA/SBUF), fp32 PSUM accumulation.
#          216 ns/MM sustained (FWL hides LDWEIGHTS), rel err ~9.6e-3.
MM_MODE = os.environ.get("BASS_LSTM_MM", "bf16")

# PE_HAM warm-up length: the PE clock gate defaults to K=4/8 (half rate)
# and only releases after ~3.4us of continuous HIGH activity.  Dummy
# full-width matmuls burn that ramp while the critical x0/w_ih[j0] DMA
# wave is in flight; the count is tuned so warmup ends right when the
# real stream can run stall-free (DMA-paced).
WARM_MMS = int(os.environ.get("BASS_LSTM_WARM", "10"))

_CACHE = {}


def _build(T, Bc, mode):
    import concourse.mybir as mybir
    import concourse.tile as tile
    from concourse import bacc

    dt = mybir.dt
    AF = mybir.ActivationFunctionType

    if mode == "bf16":
        x_dt = rec_dt = dt.bfloat16
    else:
        x_dt = rec_dt = dt.float32r

    nc = bacc.Bacc("TRN2", target_bir_lowering=False, debug=False,
                   num_devices=NCORES)
    # All inputs are host-prepped so each SBUF tile's per-partition data is
    # contiguous in DRAM (see _prep_inputs).
    xP = nc.dram_tensor("xP", [T, P, KI, Bc], x_dt, kind="ExternalInput").ap()
    w_ih = nc.dram_tensor("w_ih", [NJ, P, KI, 4 * P], x_dt,
                          kind="ExternalInput").ap()
    w_hh = nc.dram_tensor("w_hh", [NJ, P, KH, 4 * P], rec_dt,
                          kind="ExternalInput").ap()
    bias = nc.dram_tensor("bias", [P, NM], dt.float32, kind="ExternalInput").ap()
    out_dt = rec_dt
    yT = nc.dram_tensor("yT", [T, HIDDEN, Bc], out_dt, kind="ExternalOutput").ap()

    with tile.TileContext(nc) as tc:
        with tc.tile_pool(name="wpool", bufs=1) as wp, \
             tc.tile_pool(name="xpool", bufs=2) as xp, \
             tc.tile_pool(name="state", bufs=2) as sp, \
             tc.tile_pool(name="gates", bufs=2) as gp, \
             tc.tile_pool(name="tmp", bufs=2) as tp, \
             tc.tile_pool(name="psum", bufs=2, space="PSUM") as pp:

            # PE_HAM warm-up: dummy full-width matmuls over zeroed scratch
            # (short-N dummies read as low activity and re-throttle).  The
            # memsets go on the GPSIMD queue, which clears its framework
            # preamble earliest, so the ramp starts ~1.5us sooner than a
            # DVE memset would allow and doesn't occupy the DMA queues.
            if x_dt == dt.bfloat16:
                warm_w = wp.tile([P, P], x_dt, tag="warm")
                nc.gpsimd.memset(warm_w, 0.0)
                warm_x = wp.tile([P, Bc], x_dt, tag="warmx")
                nc.gpsimd.memset(warm_x, 0.0)
                warm_ps = pp.tile([P, Bc], dt.float32, tag="ps0")
                for _ in range(WARM_MMS):
                    nc.tensor.matmul(warm_ps, lhsT=warm_w, rhs=warm_x,
                                     start=True, stop=True)

            # bias rides the gpsimd SWDGE queue: tiny, needed only by the
            # first ACT (~16us), and its 128 small packets would otherwise
            # delay the critical pair transfers on a HWDGE queue.
            bias_sb = wp.tile([P, NM], dt.float32, tag="bias")
            nc.gpsimd.dma_start(out=bias_sb, in_=bias[:, :])

            # Startup ramp: t=0's x and w_ih[j0] land as 2-k-slice pair
            # tiles (2KB DMA rows) interleaved across the two HW DMA
            # queues, so the ring serves both streams k-ordered and the
            # PE chews (x,w) slices as they arrive.  (Splitting k0/k1
            # into singles was tried: the 2 extra transfers delay the
            # later pairs more than the early start gains.)
            # x0sl[k] = (tile, index into its pair dim) for each k.
            x0sl, w0sl = [None] * KI, [None] * KI

            chunks = [(0, 2), (2, 2), (4, 2), (6, 2)]
            for ci, (k0, nsl) in enumerate(chunks):
                xt_ = wp.tile([P, nsl, Bc], x_dt, tag=f"x0c{ci}")
                wt_ = wp.tile([P, nsl, 4 * P], x_dt, tag=f"w0c{ci}")
                if ci % 2 == 0:
                    nc.sync.dma_start(out=xt_, in_=xP[0, :, k0:k0 + nsl, :])
                    nc.scalar.dma_start(out=wt_, in_=w_ih[0, :, k0:k0 + nsl, :])
                else:
                    nc.scalar.dma_start(out=xt_, in_=xP[0, :, k0:k0 + nsl, :])
                    nc.sync.dma_start(out=wt_, in_=w_ih[0, :, k0:k0 + nsl, :])
                for s in range(nsl):
                    x0sl[k0 + s] = (xt_, s)
                    w0sl[k0 + s] = (wt_, s)

            # w_ih[j1..j3] are the next binding constraint (j1 must land by
            # the time the real stream reaches it, ~7us after it starts):
            # each loads as one tile, half per queue, 8KB-contiguous rows.
            w_ih_sb = [None] * NJ
            for j in range(1, NJ):
                wt_ = wp.tile([P, KI, 4 * P], x_dt, tag=f"wih{j}")
                if j < 3:
                    # j1 is the binding arrival for a stall-free stream
                    # (needed ~7us after it starts); quarter it (and j2)
                    # across the queues so its early k-slices land sooner
                    # for the t=0 k-outer consumer.  j3/x1/w_hh have slack
                    # and absorb the extra trigger-issue time.
                    nc.sync.dma_start(out=wt_[:, 0:2, :],
                                      in_=w_ih[j, :, 0:2, :])
                    nc.scalar.dma_start(out=wt_[:, 2:4, :],
                                        in_=w_ih[j, :, 2:4, :])
                    nc.sync.dma_start(out=wt_[:, 4:6, :],
                                      in_=w_ih[j, :, 4:6, :])
                    nc.scalar.dma_start(out=wt_[:, 6:8, :],
                                        in_=w_ih[j, :, 6:8, :])
                else:
                    nc.sync.dma_start(out=wt_[:, :KI // 2, :],
                                      in_=w_ih[j, :, :KI // 2, :])
                    nc.scalar.dma_start(out=wt_[:, KI // 2:, :],
                                        in_=w_ih[j, :, KI // 2:, :])
                w_ih_sb[j] = wt_

            # x1 and w_hh aren't needed until the scan reaches t=1
            # (~28us after the stream starts); they ride behind on scalar.
            xt1 = None
            if T > 1:
                xt1 = xp.tile([P, KI, Bc], x_dt, tag="x")
                nc.scalar.dma_start(out=xt1, in_=xP[1])
            w_hh_sb = []
            for j in range(NJ):
                wt_ = wp.tile([P, KH, 4 * P], rec_dt, tag=f"whh{j}")
                nc.scalar.dma_start(out=wt_, in_=w_hh[j])
                w_hh_sb.append(wt_)

            # h0 = c0 = 0, so step 0 skips the recurrent matmuls and the
            # f*c term entirely -- no initial state tiles needed.
            h_cur, c_cur = [], []

            GATE_FUNCS = (AF.Sigmoid, AF.Sigmoid, AF.Tanh, AF.Sigmoid)

            # j=0's weights live in the startup slice tiles for all t.
            def wih_ap(j, k, gi):
                if j == 0:
                    t_, s = w0sl[k]
                    return t_[:, s, gi * P:(gi + 1) * P]
                return w_ih_sb[j][:, k, gi * P:(gi + 1) * P]

            for t in range(T):
                if t == 1:
                    xt = xt1
                elif t > 1:
                    xt = xp.tile([P, KI, Bc], x_dt, tag="x")
                    if t == 2:
                        # t=2's prefetch has a free pool buffer at program
                        # start, so on sync it would transfer DURING the
                        # critical startup window and steal ring bandwidth.
                        # The scalar queue is FIFO behind x1+w_hh, which
                        # delays it to ~36us (needed ~85us).
                        nc.scalar.dma_start(out=xt, in_=xP[t])
                    else:
                        nc.sync.dma_start(out=xt, in_=xP[t])

                h_next, c_next = [], []
                for j in range(NJ):
                    acts = []
                    if t == 0:
                        # Startup step: k-OUTER, gate-inner, so the PE can
                        # consume each (x0, w0) pair slice the moment it
                        # lands -- the matmul stream rides the DMA ramp
                        # instead of waiting for the full 2MB critical set.
                        # Four PSUM banks accumulate the four gates.
                        ps_a = pp.tile([P, Bc], dt.float32, tag="ps0")
                        ps_b = pp.tile([P, Bc], dt.float32, tag="ps1")
                        ps_c = pp.tile([P, Bc], dt.float32, tag="ps2")
                        ps_d = pp.tile([P, Bc], dt.float32, tag="ps3")
                        pss = [ps_a, ps_b, ps_c, ps_d]
                        for k in range(KI):
                            xt_, xs_ = x0sl[k]
                            for gi in range(4):
                                nc.tensor.matmul(
                                    pss[gi],
                                    lhsT=wih_ap(j, k, gi),
                                    rhs=xt_[:, xs_, :],
                                    start=(k == 0), stop=(k == KI - 1))
                        for gi in range(4):
                            m = gi * NJ + j
                            gt = gp.tile([P, Bc], dt.float32, tag=f"g{gi}")
                            nc.scalar.activation(gt, pss[gi], GATE_FUNCS[gi],
                                                 bias=bias_sb[:, m:m + 1])
                            acts.append(gt)
                    else:
                        for gi in range(4):
                            m = gi * NJ + j
                            ps = pp.tile([P, Bc], dt.float32, tag=f"ps{gi}")

                            def mm_x(first):
                                for k in range(KI):
                                    nc.tensor.matmul(
                                        ps,
                                        lhsT=wih_ap(j, k, gi),
                                        rhs=xt[:, k, :],
                                        start=(first and k == 0),
                                        stop=(not first and k == KI - 1))

                            def mm_h(first):
                                for k in range(KH):
                                    nc.tensor.matmul(
                                        ps,
                                        lhsT=w_hh_sb[j][:, k,
                                                        gi * P:(gi + 1) * P],
                                        rhs=h_cur[k],
                                        start=(first and k == 0),
                                        stop=(not first and k == KH - 1))

                            # Alternate x/h group order between consecutive
                            # gate tiles so same-k-tile matmul groups chain
                            # across tile boundaries.
                            if gi % 2 == 0:
                                mm_x(True)
                                mm_h(False)
                            else:
                                mm_h(True)
                                mm_x(False)
                            gt = gp.tile([P, Bc], dt.float32, tag=f"g{gi}")
                            if t == T - 1 and gi == 3:
                                # o-gate of the last step feeds the terminal
                                # drain; halve its activation so the first
                                # half of that chain starts earlier.
                                H2 = Bc // 2
                                for s0 in (0, H2):
                                    sl = slice(s0, s0 + H2)
                                    nc.scalar.activation(
                                        gt[:, sl], ps[:, sl], GATE_FUNCS[gi],
                                        bias=bias_sb[:, m:m + 1])
                            else:
                                nc.scalar.activation(gt, ps, GATE_FUNCS[gi],
                                                     bias=bias_sb[:, m:m + 1])
                            acts.append(gt)
                    i_t, f_t, g_t, o_t = acts
                    cn = sp.tile([P, Bc], dt.float32, tag=f"c{j}")
                    th = tp.tile([P, Bc], dt.float32, tag="th")
                    hn = sp.tile([P, Bc], rec_dt, tag=f"h{j}")
                    if t == T - 1:
                        # Last step: h feeds only the output DMA.  Halve the
                        # whole u/v/c/tanh DVE+ACT pipeline so tanh(c) is
                        # done before the o-gate matmuls finish, then mul
                        # per half and store each 128x512 chunk as one DMA
                        # (full 1KB rows drain the ring faster than split
                        # halves would).
                        H2 = Bc // 2
                        u = tp.tile([P, Bc], dt.float32, tag="u")
                        v = tp.tile([P, Bc], dt.float32, tag="v")
                        for s0 in (0, H2):
                            sl = slice(s0, s0 + H2)
                            nc.vector.tensor_mul(u[:, sl], i_t[:, sl],
                                                 g_t[:, sl])
                            nc.vector.tensor_mul(v[:, sl], f_t[:, sl],
                                                 c_cur[j][:, sl])
                            nc.vector.tensor_add(cn[:, sl], u[:, sl],
                                                 v[:, sl])
                            nc.scalar.activation(th[:, sl], cn[:, sl],
                                                 AF.Tanh)
                        # Store halves on both HWDGE queues so the ring
                        # drains the final output sooner (the teardown
                        # barrier waits on it).
                        nc.vector.tensor_mul(hn[:, :H2], o_t[:, :H2],
                                             th[:, :H2])
                        nc.sync.dma_start(out=yT[t, j * P:(j + 1) * P, :H2],
                                          in_=hn[:, :H2])
                        nc.vector.tensor_mul(hn[:, H2:], o_t[:, H2:],
                                             th[:, H2:])
                        nc.scalar.dma_start(out=yT[t, j * P:(j + 1) * P, H2:],
                                            in_=hn[:, H2:])
                    else:
                        if t == 0:
                            nc.vector.tensor_mul(cn, i_t, g_t)
                        else:
                            u = tp.tile([P, Bc], dt.float32, tag="u")
                            nc.vector.tensor_mul(u, i_t, g_t)
                            v = tp.tile([P, Bc], dt.float32, tag="v")
                            nc.vector.tensor_mul(v, f_t, c_cur[j])
                            nc.vector.tensor_add(cn, u, v)
                        nc.scalar.activation(th, cn, AF.Tanh)
                        nc.vector.tensor_mul(hn, o_t, th)
                        nc.sync.dma_start(out=yT[t, j * P:(j + 1) * P, :],
                                          in_=hn)
                    h_next.append(hn)
                    c_next.append(cn)
                h_cur, c_cur = h_next, c_next

    nc.compile()
    return nc


def _get_nc(T=FRAME_LENGTH, Bc=BC, mode=MM_MODE):
    key = (T, Bc, mode, WARM_MMS)
    if key not in _CACHE:
        _CACHE[key] = _build(T, Bc, mode)
    return _CACHE[key]


def _prep_inputs(embed_feats, w_ih_l, w_hh_l, b_ih_l, b_hh_l,
                 w_ih_r, w_hh_r, b_ih_r, b_hh_r, mode):
    import ml_dtypes

    if mode == "bf16":
        x_np = rec_np = ml_dtypes.bfloat16
    else:
        x_np = rec_np = np.float32
    T = embed_feats.shape[1]

    w = {
        0: (np.asarray(w_ih_l), np.asarray(w_hh_l),
            np.asarray(b_ih_l) + np.asarray(b_hh_l)),
        1: (np.asarray(w_ih_r), np.asarray(w_hh_r),
            np.asarray(b_ih_r) + np.asarray(b_hh_r)),
    }
    x = np.asarray(embed_feats)

    # j-major column permutation of the 4H gate dim: block j holds the four
    # gates' columns for hidden chunk j, so each j-chunk loads contiguously
    j_idx, g_idx, c_idx = np.meshgrid(
        np.arange(NJ), np.arange(4), np.arange(P), indexing="ij")
    perm = (g_idx * (NJ * P) + j_idx * P + c_idx).reshape(-1)

    in_maps = []
    for c in range(NCORES):
        d, s = c // NSHARD, c % NSHARD
        xs = x[s * BC:(s + 1) * BC]
        if d == 1:
            xs = xs[:, ::-1]
        # [Bc, T, I] -> [T, P, KI, Bc]: per-(t, partition) data contiguous
        xT = xs.transpose(1, 2, 0).reshape(T, KI, P, BC).transpose(0, 2, 1, 3)
        xPh = np.ascontiguousarray(xT).astype(x_np)
        # w_ih.T[:, perm]: [I, 4H] -> [NJ, P, KI, 4P] (j, partition)-major
        w_ihT = w[d][0].T[:, perm].reshape(KI, P, NJ, 4 * P).transpose(2, 1, 0, 3)
        w_ihT = np.ascontiguousarray(w_ihT).astype(x_np)
        w_hhT = w[d][1].T[:, perm].reshape(KH, P, NJ, 4 * P).transpose(2, 1, 0, 3)
        w_hhT = np.ascontiguousarray(w_hhT).astype(rec_np)
        bias = np.ascontiguousarray(
            w[d][2].astype(np.float32).reshape(NM, P).T)
        in_maps.append({"xP": xPh, "w_ih": w_ihT, "w_hh": w_hhT, "bias": bias})
    return in_maps, T


def _run(inputs, mode=MM_MODE, trace=False, trace_kwargs=None):
    from concourse.bass_utils import run_bass_kernel_spmd

    in_maps, T = _prep_inputs(mode=mode, **inputs)
    nc = _get_nc(T=T, mode=mode)
    res = run_bass_kernel_spmd(nc, in_maps, list(range(NCORES)),
                               trace=trace, **(trace_kwargs or {}))

    out = np.empty((BATCH, T, 2 * HIDDEN), np.float32)
    for c in range(NCORES):
        d, s = c // NSHARD, c % NSHARD
        yt = np.asarray(res.results[c]["yT"], dtype=np.float32)  # [T, H, Bc]
        arr = yt.transpose(2, 0, 1)                              # [Bc, T, H]
        if d == 1:
            arr = arr[:, ::-1]
        out[s * BC:(s + 1) * BC, :, d * HIDDEN:(d + 1) * HIDDEN] = arr
    return out, res


def kernel(**inputs):
    out, _ = _run(inputs)
    return out
